# revision 1
# baseline (speedup 1.0000x reference)
"""Trainium2 Bass kernel for the NEUROPULS unitary NxN photonic mesh.

Parallel-scan reformulation. The reference chain is 128 sequential
structured steps X <- CR@MMI@diag(p_it)@MMI@X (last step without CR),
starting from X = diag(p_0) and finishing with a diag(p_129) row scale;
the output is the accumulated 256x256 complex matrix.

Instead of running 128 latency-bound steps on every core, each core m
computes the *group product* G_m = A_{16m+16}...A_{16m+1} of its 16
structured factors as a band-packed matrix (band +-32, 76 stored
diagonals, fp32), using the same E-step/CR-step pair-layout machinery as
the direct method -- G starts as (packed) identity, so a group step costs
the same as a direct step but on a ~65-wide band instead of 32 columns.
diag(p_0) folds into core 0's G init, diag(p_129) into core 7's
post-scale, and the missing final crossing is an identity-CR via per-step
blend masks, so the SPMD program is uniform across cores.

The 8 packed G's are AllGather'ed (69.6KB/core, fp16), scattered into
zero-backed DRAM strips, and densified for free by reading them back with
a skewed access pattern (stride 511 over rows): row r's band lands at
dense columns [r-32, r+32], everything else reads pre-zeroed margin.
Each core then redundantly computes F^T = G_0^T G_1^T ... G_7^T as two
independent 4-multiply half-chains that interleave on PE/DVE (lhsT = G_k
row-major, rhs = running product; PSUM accumulation over row blocks and
the complex cross terms, with a pre-negated imaginary weight plane built
on GpSimd), joined by a PE transpose of the left half and a final
multiply. The host transposes F^T into the full output.
"""

import numpy as np

import bass_rust
import concourse.bass as bass
import concourse.mybir as mybir
import concourse.tile as tile
from concourse.bass_utils import run_bass_kernel_spmd

N = 256
NCORES = 8
GITS = 16          # iterations per group
J = 76             # packed band width (diag offsets)
W = 36             # packed center: G[r, j] = G_dense[r, r + j - W]
SP = 512           # strip pitch (elements) in the zero-backed skew DRAM
S0 = 224           # strip start offset within a row's pitch

IL_MMI = 0.02
IMB = 0.01
IL_CR = 0.02
CT = 0.01

_A_MMI = float(np.sqrt(1.0 - IL_MMI))
AT = _A_MMI * float(np.sqrt((1.0 + IMB) / 2.0))
AR = _A_MMI * float(np.sqrt((1.0 - IMB) / 2.0))
_A_CR = float(np.sqrt(1.0 - IL_CR))
G1S = _A_CR * float(np.sqrt(CT))        # CR diag (mid rows)
G2C = _A_CR * float(np.sqrt(1.0 - CT))  # CR off-diag (x i); also thru

F32 = mybir.dt.float32
F16 = mybir.dt.float16
MULT = mybir.AluOpType.mult
ADD = mybir.AluOpType.add
SUB = mybir.AluOpType.subtract
SIN = mybir.ActivationFunctionType.Sin
PI = float(np.pi)

_ENGINE_SEM_PREFIXES = {
    "DVE": ("DVE_",),
    "ACT": ("ACT_", "Activation_"),
    "PE": ("PE_",),
    "POOL": ("Pool_", "POOL_"),
    "SP": ("SP_",),
}


def strip_same_engine_waits(nc):
    for bb in nc.main_func.blocks:
        for ins in bb.instructions:
            si = getattr(ins, "sync_info", None)
            if si is None:
                continue
            eng = getattr(ins, "engine", None)
            pres = _ENGINE_SEM_PREFIXES.get(getattr(eng, "name", ""), ())
            if not pres:
                continue
            kept = [
                w
                for w in si.on_wait
                if not (
                    w.sync_type == "semaphore"
                    and w.ant_name
                    and w.ant_name.startswith(pres)
                )
            ]
            if len(kept) != len(si.on_wait):
                si.on_wait = kept
                ins.sync_info = si


def split_multi_waits(nc):
    """This walrus build allows one sync-wait per instruction: hoist extra
    waits onto same-engine Drain nops inserted just before the instruction."""
    for bb in nc.main_func.blocks:
        insts = bb.instructions
        i = 0
        while i < len(insts):
            ins = insts[i]
            si = getattr(ins, "sync_info", None)
            if si is None or len(si.on_wait) <= 1:
                i += 1
                continue
            waits = list(si.on_wait)
            for k, w in enumerate(waits[:-1]):
                d = mybir.InstDrain(name=f"{ins.name}_waitsplit{k}", ins=[], outs=[])
                d.engine = ins.engine
                d.sync_info = bass_rust.SyncInfo(on_wait=[w], on_update=[])
                insts.insert(i, d)
                i += 1
            si.on_wait = [waits[-1]]
            ins.sync_info = si
            i += 1


def fix_sync_waits(nc):
    split_multi_waits(nc)


def _skew_ap(strips, k, rb):
    """Dense row-major read of row-block rb of G_k (both planes) from the
    zero-backed strip area: element (r, c) of plane pl at strip offset
    r*SP + S0 + (c - r + W).  Dims: [r-part 128, pl 2, c 256]."""
    ap = strips[:]
    base = k * (2 * 256 * SP) + rb * 128 * (SP - 1) + (S0 + W)
    ap.ap = bass_rust.VecI64Pair([[SP - 1, 128], [256 * SP, 2], [1, 256]])
    ap.offset = base
    return ap


def build_nc(nsteps=GITS):
    nc = bass.Bass(num_devices=8)

    thg = nc.dram_tensor("thg", [18, N], F32, kind="ExternalInput")
    isg0 = nc.dram_tensor("isg0", [128, 1], F32, kind="ExternalInput")
    postm = nc.dram_tensor("postm", [128, 1], F32, kind="ExternalInput")
    gmd = nc.dram_tensor("gmd", [128, GITS], F32, kind="ExternalInput")
    gescd = nc.dram_tensor("gescd", [128, GITS, 2], F32, kind="ExternalInput")
    wconst = nc.dram_tensor("wconst", [4, 128, 128], F32, kind="ExternalInput")
    permw = nc.dram_tensor("permw", [4, 128, 128], F32, kind="ExternalInput")
    idmd = nc.dram_tensor("idmd", [128, 2, 256], F16, kind="ExternalInput")
    out_d = nc.dram_tensor("out", [128, 2, 2, 256], F16, kind="ExternalOutput")

    JB = 68  # shipped band slice: packed j in [4, 4+JB)
    gsend = nc.dram_tensor("gsend", [256, 2, 68], F16, kind="Internal")
    gall = nc.dram_tensor("gall", [8, 256, 2, 68], F16, kind="Internal")
    strips = nc.dram_tensor("strips", [8, 2, 256, SP], F16, kind="Internal")

    with tile.TileContext(nc) as tc:
        with (
            tc.tile_pool(name="coef", bufs=1) as cp,
            tc.tile_pool(name="state", bufs=1) as sp,
            tc.tile_pool(name="mchain", bufs=2) as mp,
            tc.tile_pool(name="lts", bufs=2) as lp,
            tc.tile_pool(name="psum", bufs=2, space="PSUM") as pp,
        ):
            # ---------------- setup: trig + step coefficients ----------------
            th = cp.tile([128, 18, 2], F32, tag="th")
            Ct = cp.tile([128, 18, 2], F32, tag="Ct")
            St = cp.tile([128, 18, 2], F32, tag="St")
            wrk = cp.tile([128, 18, 2], F32, tag="wrk")
            wrp = cp.tile([128, 18, 2], F32, tag="wrp")
            zb = cp.tile([128, 1], F32, tag="zb")
            d1r = cp.tile([128, GITS, 2], F32, tag="d1r")
            d1i = cp.tile([128, GITS, 2], F32, tag="d1i")
            d1iN = cp.tile([128, GITS, 2], F32, tag="d1iN")
            d2r = cp.tile([128, GITS, 2], F32, tag="d2r")
            d2i = cp.tile([128, GITS, 2], F32, tag="d2i")
            isg = cp.tile([128, 1], F32, tag="isg")
            psm = cp.tile([128, 1], F32, tag="psm")
            gmt = cp.tile([128, GITS], F32, tag="gmt")
            gt = cp.tile([128, GITS, 2], F32, tag="gt")
            Wt = cp.tile([128, 4, 128], F32, tag="Wt")
            Pt = cp.tile([128, 4, 128], F32, tag="Pt")
            idt = cp.tile([128, 2, 256], F16, tag="idt")

            nc.sync.dma_start(th[:], thg[:].rearrange("it (k e) -> k it e", k=128, e=2))
            nc.sync.dma_start(isg[:], isg0[:])
            nc.sync.dma_start(psm[:], postm[:])
            nc.sync.dma_start(gmt[:], gmd[:])
            nc.sync.dma_start(gt[:], gescd[:])
            nc.sync.dma_start(Wt[:], wconst[:].rearrange("w p f -> p w f"))
            nc.sync.dma_start(Pt[:], permw[:].rearrange("w p f -> p w f"))
            nc.sync.dma_start(idt[:], idmd[:])
            nc.vector.memset(zb[:], 0.0)

            # zero-fill the skew strips (after the inputs so they don't
            # delay setup; overlaps phase A on the DMA engines)
            zt = cp.tile([128, 2048], F16, tag="zt")
            nc.vector.memset(zt[:], 0.0)
            for g in range(8):
                dst = strips[g].rearrange(
                    "pl (pa pb) c -> (pl pa) (pb c)", pa=64, pb=4
                )
                nc.gpsimd.dma_start(dst, zt[:])

            # sin/cos with range reduction into (-pi, pi]
            nc.vector.tensor_scalar(wrp[:], th[:], PI, -2 * PI, mybir.AluOpType.is_gt, MULT)
            nc.vector.tensor_tensor(wrk[:], th[:], wrp[:], ADD)
            nc.scalar.activation(St[:], wrk[:], SIN, bias=zb[:])
            nc.vector.tensor_scalar(wrk[:], th[:], PI / 2, None, ADD)
            nc.vector.tensor_scalar(wrp[:], wrk[:], PI, -2 * PI, mybir.AluOpType.is_gt, MULT)
            nc.vector.tensor_tensor(wrk[:], wrk[:], wrp[:], ADD)
            nc.scalar.activation(Ct[:], wrk[:], SIN, bias=zb[:])

            Cmid = Ct[:, :GITS, :]
            Smid = St[:, :GITS, :]
            Csw = Ct[:, :GITS, ::-1]
            Ssw = St[:, :GITS, ::-1]
            wmid = wrk[:, :GITS, :]

            # d1 = at^2 p - ar^2 p^sigma ; d2 = i at ar (p + p^sigma)
            nc.vector.tensor_scalar(wmid, Csw, -AR * AR, None, MULT)
            nc.vector.scalar_tensor_tensor(d1r[:], Cmid, AT * AT, wmid, MULT, ADD)
            nc.vector.tensor_scalar(wmid, Ssw, -AR * AR, None, MULT)
            nc.vector.scalar_tensor_tensor(d1i[:], Smid, AT * AT, wmid, MULT, ADD)
            nc.vector.tensor_scalar(d1iN[:], d1i[:], -1.0, None, MULT)
            nc.vector.tensor_tensor(wmid, Smid, Ssw, ADD)
            nc.vector.tensor_scalar(d2r[:], wmid, -AT * AR, None, MULT)
            nc.vector.tensor_tensor(wmid, Cmid, Csw, ADD)
            nc.vector.tensor_scalar(d2i[:], wmid, AT * AR, None, MULT)

            # p129 post-scale blend: ceff = postm*c129 + (1-postm); seff = postm*s129
            ceff = cp.tile([128, 2], F32, tag="ceff")
            seff = cp.tile([128, 2], F32, tag="seff")
            seffN = cp.tile([128, 2], F32, tag="seffN")
            npsm = cp.tile([128, 1], F32, tag="npsm")
            nc.vector.tensor_scalar(npsm[:], psm[:], -1.0, 1.0, MULT, ADD)
            for e in range(2):
                nc.vector.scalar_tensor_tensor(
                    ceff[:, e : e + 1], Ct[:, 17, e : e + 1], psm[:], npsm[:], MULT, ADD
                )
                nc.vector.tensor_scalar(seff[:, e : e + 1], St[:, 17, e : e + 1], psm[:], None, MULT)
            nc.vector.tensor_scalar(seffN[:], seff[:], -1.0, None, MULT)

            # ---------------- phase A state ----------------
            Gpp_a = sp.tile([128, 2, 2, J], F32, tag="Ga")
            Gpp_b = sp.tile([128, 2, 2, J], F32, tag="Gb")
            Gpp = [Gpp_a, Gpp_b]
            G = Gpp[0]
            V = sp.tile([128, 2, 2, J], F32, tag="V")
            Ypp_a = sp.tile([128, 2, 2, J], F32, tag="Ya")
            Ypp_b = sp.tile([128, 2, 2, J], F32, tag="Yb")
            Ypp = [Ypp_a, Ypp_b]
            Y = Ypp[0]
            U = sp.tile([128, 2, 2, J], F32, tag="U")
            Ysh = sp.tile([128, 2, 2, 80], F32, tag="Ysh")
            TGT = sp.tile([128, 2, 2, J], F32, tag="TGT")
            T0 = sp.tile([128, 2, J], F32, tag="T0")
            T1 = sp.tile([128, 2, J], F32, tag="T1")
            for i in range(2):
                nc.vector.memset(Gpp[i][:], 0.0)
                nc.vector.memset(Ypp[i][:], 0.0)
            nc.vector.memset(V[:], 0.0)
            nc.vector.memset(U[:], 0.0)

            # G init: identity (or diag(p0) on core 0): G[p,e,:,W]
            nm0 = cp.tile([128, 1], F32, tag="nm0")
            nc.vector.tensor_scalar(nm0[:], isg[:], -1.0, 1.0, MULT, ADD)
            for e in range(2):
                nc.vector.scalar_tensor_tensor(
                    G[:, e, 0, W : W + 1], Ct[:, 16, e : e + 1], isg[:], nm0[:], MULT, ADD
                )
                nc.vector.tensor_scalar(
                    G[:, e, 1, W : W + 1], St[:, 16, e : e + 1], isg[:], None, MULT
                )


            # ---------------- phase A: 16 group steps ----------------
            for s in range(nsteps):
                G = Gpp[s % 2]
                Gout = Gpp[(s + 1) % 2]
                Y = Ypp[s % 2]
                he = 2 * s + 1   # E-step half-width
                hc = 2 * s + 2   # CR half-width

                def sl(h, d=0):
                    return slice(W - h + d, W + h + 1 + d)

                cd1r = [d1r[:, s, e : e + 1] for e in range(2)]
                cd1i = [d1i[:, s, e : e + 1] for e in range(2)]
                cd1iN = [d1iN[:, s, e : e + 1] for e in range(2)]
                cd2r = d2r[:, s, 0:1]
                cd2i = d2i[:, s, 0:1]

                # V = i*G on the CR window (reads beyond G's band are zeros)
                nc.vector.tensor_scalar(V[:, :, 0, sl(hc)], G[:, :, 1, sl(hc)], -1.0, None, MULT)
                nc.vector.tensor_copy(out=V[:, :, 1, sl(hc)], in_=G[:, :, 0, sl(hc)])

                # u[e] = d2 * G[1-e] (j-shifted -1 for e=0, +1 for e=1)
                nc.vector.tensor_scalar(T0[:, :, sl(he)], V[:, 1, :, sl(he, -1)], cd2i, None, MULT)
                nc.vector.scalar_tensor_tensor(
                    U[:, 0, :, sl(he)], G[:, 1, :, sl(he, -1)], cd2r, T0[:, :, sl(he)], MULT, ADD
                )
                nc.vector.tensor_scalar(T1[:, :, sl(he)], V[:, 0, :, sl(he, +1)], cd2i, None, MULT)
                nc.vector.scalar_tensor_tensor(
                    U[:, 1, :, sl(he)], G[:, 0, :, sl(he, +1)], cd2r, T1[:, :, sl(he)], MULT, ADD
                )

                # Y[e] = d1r[e]*G[e] + d1i[e]*V[e] + u[e]  (two-plane fused).
                # Stored pre-shifted per e (e=0 at j-1, e=1 at j+1) so the
                # crossing matmuls read unshifted 16B-aligned windows.
                for e in range(2):
                    sh = +1 if e == 0 else -1
                    nc.vector.scalar_tensor_tensor(
                        Y[:, e, :, sl(he, sh)], V[:, e, :, sl(he)], cd1i[e], U[:, e, :, sl(he)], MULT, ADD
                    )
                    nc.vector.scalar_tensor_tensor(
                        Y[:, e, :, sl(he, sh)], G[:, e, :, sl(he)], cd1r[e], Y[:, e, :, sl(he, sh)], MULT, ADD
                    )

                # CR: sgP[p,0] = (+-G2C)*Y[p-1,1] at j+1; sgP[p,1] = (+-G2C)*Y[p+1,0] at j-1
                # Shift on DVE (any offset), then matmul on 16B-aligned windows.
                lo4 = ((W - hc) // 4) * 4
                hi4 = ((W + hc + 1 + 3) // 4) * 4
                al = slice(lo4, hi4)
                alp = slice(lo4 + 1, hi4 + 1)
                alm = slice(lo4 - 1, hi4 - 1)
                sgP = pp.tile([128, 2, 2, J], F32, tag="sgP")
                nc.tensor.matmul(sgP[:, 0, 0, al], Wt[:, 0, :], Y[:, 1, 1, al], start=True, stop=True)
                nc.tensor.matmul(sgP[:, 0, 1, al], Wt[:, 1, :], Y[:, 1, 0, al], start=True, stop=True)
                nc.tensor.matmul(sgP[:, 1, 0, al], Wt[:, 2, :], Y[:, 0, 1, al], start=True, stop=True)
                nc.tensor.matmul(sgP[:, 1, 1, al], Wt[:, 3, :], Y[:, 0, 0, al], start=True, stop=True)

                # tg[e] = gesc[s,e] * Y[e]  (ACT, off critical path)
                nc.vector.tensor_scalar(TGT[:, 0, :, sl(hc)], Y[:, 0, :, sl(hc, +1)], gt[:, s, 0:1], None, MULT)
                nc.scalar.mul(TGT[:, 1, :, sl(hc)], Y[:, 1, :, sl(hc, -1)], gt[:, s, 1:2])

                # G' = gm[s]*sgP + tg  (single 4-plane op; esc lives in TGT)
                nc.vector.scalar_tensor_tensor(
                    Gout[:, :, :, sl(hc)], sgP[:, :, :, sl(hc)], gmt[:, s : s + 1],
                    TGT[:, :, :, sl(hc)], MULT, ADD,
                )

            # ---------------- p129 post-scale (core 7; identity elsewhere) ---
            G = Gpp[nsteps % 2]
            fb = slice(W - 32, W + 33)
            for e in range(2):
                nc.vector.tensor_scalar(T0[:, e, fb], G[:, e, 1, fb], seffN[:, e : e + 1], None, MULT)
                nc.vector.tensor_scalar(T1[:, e, fb], G[:, e, 1, fb], ceff[:, e : e + 1], None, MULT)
                nc.vector.scalar_tensor_tensor(
                    G[:, e, 1, fb], G[:, e, 0, fb], seff[:, e : e + 1], T1[:, e, fb], MULT, ADD
                )
                nc.vector.scalar_tensor_tensor(
                    G[:, e, 0, fb], G[:, e, 0, fb], ceff[:, e : e + 1], T0[:, e, fb], MULT, ADD
                )


            # ---------------- pair -> linear row permutation ----------------
            pp2 = pp.tile([128, 2, 2, J], F32, tag="pp2", bufs=1)
            for rb in range(2):
                nc.tensor.matmul(pp2[:, rb, :, :], Pt[:, 2 * rb + 0, :], G[:, 0, :, :], start=True, stop=False)
                nc.tensor.matmul(pp2[:, rb, :, :], Pt[:, 2 * rb + 1, :], G[:, 1, :, :], start=False, stop=True)
            glin = sp.tile([128, 2, 2, 68], F16, tag="glin")
            nc.vector.tensor_copy(out=glin[:], in_=pp2[:, :, :, 4:72])

            # ---------------- stage out + AllGather + strip scatter ----------
            nc.sync.dma_start(
                gsend[:].rearrange("(rb p) ri j -> p rb ri j", rb=2, p=128),
                glin[:],
            )
            nc.gpsimd.collective_compute(
                "AllGather", mybir.AluOpType.bypass,
                replica_groups=[[0, 1, 2, 3, 4, 5, 6, 7]],
                ins=[gsend[:]],
                outs=[gall[:]],
            )
            ORDER = (7, 3, 6, 2, 5, 1, 4, 0)  # chain consumption order

            # ---------------- F^T chain, split halves ------------------------
            # R-half: Mr = G4^T G5^T G6^T G7^T ; L-half: Ml = G0^T G1^T G2^T G3^T
            # (independent 4-multiply chains that interleave on PE/DVE), then
            # join F^T = Ml . Mr via a PE transpose of Ml and one last multiply.
            Mout = sp.tile([128, 2, 2, 256], F16, tag="Mout")

            # keep PE clocked up through the collective: ~26us of dummy
            # matmuls on stale data (PE is otherwise idle and would drop to
            # the mid pstate, halving F-chain matmul speed)
            warm = pp.tile([128, 2, 2, J], F32, tag="sgP")
            for _w in range(60):
                nc.tensor.matmul(
                    warm[:].rearrange('p a b c -> p (a b c)')[:, 0:256],
                    Pt[:, 0, :],
                    Pt[:, 0:2, :].rearrange('p w f -> p (w f)'),
                    start=True, stop=True,
                )

            lts = [None] * 8
            ltIns = [None] * 8
            for idx, k in enumerate(ORDER):
                q = nc.sync if idx % 2 == 0 else nc.scalar
                q.dma_start(
                    strips[k, :, :, S0 + 4 : S0 + 4 + 68],
                    gall[k].rearrange("r pl j -> pl r j"),
                )
                ltk = lp.tile([128, 2, 2, 256], F16, tag=f"ltk{k}", bufs=1)
                for rb in range(2):
                    q.dma_start(ltk[:, rb, :, :], _skew_ap(strips, k, rb))
                lts[k] = ltk
                lik = lp.tile([128, 2, 256], F16, tag=f"ltIn{k}", bufs=1)
                nc.gpsimd.tensor_scalar(lik[:, 0, :], ltk[:, 0, 1, :], -1.0, None, MULT)
                nc.gpsimd.tensor_scalar(lik[:, 1, :], ltk[:, 1, 1, :], -1.0, None, MULT)
                ltIns[k] = lik

            Mcurs = {}
            for half, tg_ in ((0, "Mr"), (1, "Ml")):
                Mc = mp.tile([128, 2, 2, 256], F16, tag=tg_)
                nc.vector.tensor_copy(out=Mc[:, 0, 0, :], in_=idt[:, 0, :])
                nc.vector.tensor_copy(out=Mc[:, 1, 0, :], in_=idt[:, 1, :])
                nc.vector.memset(Mc[:, 0, 1, :], 0.0)
                nc.vector.memset(Mc[:, 1, 1, :], 0.0)
                Mcurs[half] = Mc

            def one_mult(Mcur, k, half, last=False):
                hs = "r" if half == 0 else "l"
                lt = lts[k]
                ltIn = ltIns[k]
                pR = pp.tile([128, 2, 256], F32, tag=f"pR{hs}", bufs=1)
                pI = pp.tile([128, 2, 256], F32, tag=f"pI{hs}", bufs=1)
                for ab in range(2):
                    abv = slice(128 * ab, 128 * (ab + 1))
                    nc.tensor.matmul(pR[:, ab, :], lt[:, 0, 0, abv], Mcur[:, 0, 0, :], start=True, stop=False)
                    nc.tensor.matmul(pR[:, ab, :], lt[:, 1, 0, abv], Mcur[:, 1, 0, :], start=False, stop=False)
                    nc.tensor.matmul(pR[:, ab, :], ltIn[:, 0, abv], Mcur[:, 0, 1, :], start=False, stop=False)
                    nc.tensor.matmul(pR[:, ab, :], ltIn[:, 1, abv], Mcur[:, 1, 1, :], start=False, stop=True)
                    nc.tensor.matmul(pI[:, ab, :], lt[:, 0, 1, abv], Mcur[:, 0, 0, :], start=True, stop=False)
                    nc.tensor.matmul(pI[:, ab, :], lt[:, 1, 1, abv], Mcur[:, 1, 0, :], start=False, stop=False)
                    nc.tensor.matmul(pI[:, ab, :], lt[:, 0, 0, abv], Mcur[:, 0, 1, :], start=False, stop=False)
                    nc.tensor.matmul(pI[:, ab, :], lt[:, 1, 0, abv], Mcur[:, 1, 1, :], start=False, stop=True)
                Mnew = mp.tile([128, 2, 2, 256], F16, tag=f"M{hs}")
                nc.vector.tensor_copy(out=Mnew[:, :, 0, :], in_=pR[:])
                nc.vector.tensor_copy(out=Mnew[:, :, 1, :], in_=pI[:])
                return Mnew

            for step in range(4):
                Mcurs[0] = one_mult(Mcurs[0], 7 - step, 0)
                Mcurs[1] = one_mult(Mcurs[1], 3 - step, 1)
            Mr, Ml = Mcurs[0], Mcurs[1]

            # transpose Ml -> lhsT form (LT[cb-part, plane-variant, a]) with a
            # negated I-plane; identity block for PE transpose = idt[:, 0, 0:128]
            ident = idt[:, 0, 0:128]
            LT = sp.tile([128, 2, 3, 256], F16, tag="LT")  # [cb, (R, I, In), a]
            for cb in range(2):
                for ab in range(2):
                    abv = slice(128 * ab, 128 * (ab + 1))
                    ptt = pp.tile([128, 2, 2, 128], F16, tag="ptt", bufs=1)
                    sl2 = ab % 2
                    nc.tensor.transpose(ptt[:, sl2, 0, :], Ml[:, ab, 0, 128 * cb : 128 * (cb + 1)], ident)
                    nc.tensor.transpose(ptt[:, sl2, 1, :], Ml[:, ab, 1, 128 * cb : 128 * (cb + 1)], ident)
                    nc.vector.tensor_copy(out=LT[:, cb, 0, abv], in_=ptt[:, sl2, 0, :])
                    nc.vector.tensor_copy(out=LT[:, cb, 1, abv], in_=ptt[:, sl2, 1, :])
                    nc.vector.tensor_scalar(LT[:, cb, 2, abv], ptt[:, sl2, 1, :], -1.0, None, MULT)

            # final multiply: Mout = Ml . Mr  (lhsT = Ml^T = LT)
            pRf = pp.tile([128, 2, 256], F32, tag="pRr", bufs=1)
            pIf = pp.tile([128, 2, 256], F32, tag="pIr", bufs=1)
            for ab in range(2):
                abv = slice(128 * ab, 128 * (ab + 1))
                nc.tensor.matmul(pRf[:, ab, :], LT[:, 0, 0, abv], Mr[:, 0, 0, :], start=True, stop=False)
                nc.tensor.matmul(pRf[:, ab, :], LT[:, 1, 0, abv], Mr[:, 1, 0, :], start=False, stop=False)
                nc.tensor.matmul(pRf[:, ab, :], LT[:, 0, 2, abv], Mr[:, 0, 1, :], start=False, stop=False)
                nc.tensor.matmul(pRf[:, ab, :], LT[:, 1, 2, abv], Mr[:, 1, 1, :], start=False, stop=True)
                nc.tensor.matmul(pIf[:, ab, :], LT[:, 0, 1, abv], Mr[:, 0, 0, :], start=True, stop=False)
                nc.tensor.matmul(pIf[:, ab, :], LT[:, 1, 1, abv], Mr[:, 1, 0, :], start=False, stop=False)
                nc.tensor.matmul(pIf[:, ab, :], LT[:, 0, 0, abv], Mr[:, 0, 1, :], start=False, stop=False)
                nc.tensor.matmul(pIf[:, ab, :], LT[:, 1, 0, abv], Mr[:, 1, 1, :], start=False, stop=True)
            nc.vector.tensor_copy(out=Mout[:, :, 0, :], in_=pRf[:])
            nc.sync.dma_start(out_d[:, :, 0, :], Mout[:, :, 0, :])
            nc.vector.tensor_copy(out=Mout[:, :, 1, :], in_=pIf[:])
            nc.sync.dma_start(out_d[:, :, 1, :], Mout[:, :, 1, :])


    return nc


def make_inputs(core: int, thetas: np.ndarray):
    m = core
    thg = np.concatenate(
        [thetas[16 * m + 1 : 16 * m + 17], thetas[0:1], thetas[129:130]], axis=0
    ).astype(np.float32)
    isg0 = np.full((128, 1), 1.0 if m == 0 else 0.0, np.float32)
    postm = np.full((128, 1), 1.0 if m == 7 else 0.0, np.float32)
    gm = np.ones((128, GITS), np.float32)
    gesc = np.full((128, GITS, 2), G1S, np.float32)
    gesc[0, :, 0] = G2C
    gesc[127, :, 1] = G2C
    if m == 7:
        gm[:, GITS - 1] = 0.0
        gesc[:, GITS - 1, :] = 1.0

    wdn = np.eye(128, k=1, dtype=np.float32)
    wup = np.eye(128, k=-1, dtype=np.float32)
    wconst = np.stack([-G2C * wdn, G2C * wdn, -G2C * wup, G2C * wup]).astype(np.float32)

    permw = np.zeros((4, 128, 128), np.float32)
    for rb in range(2):
        for e in range(2):
            for p in range(64 * rb, 64 * rb + 64):
                permw[2 * rb + e, p, 2 * p + e - 128 * rb] = 1.0

    idm = np.zeros((128, 2, 256), np.float16)
    for rb in range(2):
        for p in range(128):
            idm[p, rb, rb * 128 + p] = 1.0

    return {
        "thg": thg, "isg0": isg0, "postm": postm, "gmd": gm, "gescd": gesc,
        "wconst": wconst, "permw": permw, "idmd": idm,
    }


_CACHE = {}


def _get_nc():
    if "nc" not in _CACHE:
        nc = build_nc()
        fix_sync_waits(nc)
        _CACHE["nc"] = nc
    return _CACHE["nc"]


def _run(thetas: np.ndarray, trace: bool = False):
    thetas = np.ascontiguousarray(thetas, dtype=np.float32)
    assert thetas.shape == (130, N)
    nc = _get_nc()
    in_maps = [make_inputs(c, thetas) for c in range(NCORES)]
    res = run_bass_kernel_spmd(nc, in_maps, list(range(NCORES)), trace=trace)
    o = res.results[0]["out"]  # [128, 2(rb), 2(RI), 256] = F^T row-major
    FT = (o[:, :, 0, :] + 1j * o[:, :, 1, :]).astype(np.complex64)
    FT = FT.transpose(1, 0, 2).reshape(256, 256)  # [a, c]
    return FT.T.copy(), res


def kernel(thetas: np.ndarray) -> np.ndarray:
    out, _ = _run(thetas, trace=False)
    return out



# revision 6
# speedup vs baseline: 1.1626x; 1.1626x over previous
"""Trainium2 Bass kernel for the NEUROPULS unitary NxN photonic mesh.

Parallel-scan reformulation. The reference chain is 128 sequential
structured steps X <- CR@MMI@diag(p_it)@MMI@X (last step without CR),
starting from X = diag(p_0) and finishing with a diag(p_129) row scale;
the output is the accumulated 256x256 complex matrix.

Instead of running 128 latency-bound steps on every core, each core m
computes the *group product* G_m = A_{16m+16}...A_{16m+1} of its 16
structured factors as a band-packed matrix (band +-32, 76 stored
diagonals, fp32), using the same E-step/CR-step pair-layout machinery as
the direct method -- G starts as (packed) identity, so a group step costs
the same as a direct step but on a ~65-wide band instead of 32 columns.
diag(p_0) folds into core 0's G init, diag(p_129) into core 7's
post-scale, and the missing final crossing is an identity-CR via per-step
blend masks, so the SPMD program is uniform across cores.

The 8 packed G's are AllGather'ed (69.6KB/core, fp16), scattered into
zero-backed DRAM strips, and densified for free by reading them back with
a skewed access pattern (stride 511 over rows): row r's band lands at
dense columns [r-32, r+32], everything else reads pre-zeroed margin.
Each core then redundantly computes F^T = G_0^T G_1^T ... G_7^T as two
independent 4-multiply half-chains that interleave on PE/DVE (lhsT = G_k
row-major, rhs = running product; PSUM accumulation over row blocks and
the complex cross terms, with a pre-negated imaginary weight plane built
on GpSimd), joined by a PE transpose of the left half and a final
multiply. The host transposes F^T into the full output.
"""

import numpy as np

import bass_rust
import concourse.bass as bass
import concourse.mybir as mybir
import concourse.tile as tile
from concourse.bass_utils import run_bass_kernel_spmd

N = 256
NCORES = 8
GITS = 16          # iterations per group
J = 76             # packed band width (diag offsets)
W = 36             # packed center: G[r, j] = G_dense[r, r + j - W]
SP = 512           # strip pitch (elements) in the zero-backed skew DRAM
S0 = 224           # strip start offset within a row's pitch

IL_MMI = 0.02
IMB = 0.01
IL_CR = 0.02
CT = 0.01

_A_MMI = float(np.sqrt(1.0 - IL_MMI))
AT = _A_MMI * float(np.sqrt((1.0 + IMB) / 2.0))
AR = _A_MMI * float(np.sqrt((1.0 - IMB) / 2.0))
_A_CR = float(np.sqrt(1.0 - IL_CR))
G1S = _A_CR * float(np.sqrt(CT))        # CR diag (mid rows)
G2C = _A_CR * float(np.sqrt(1.0 - CT))  # CR off-diag (x i); also thru

F32 = mybir.dt.float32
F16 = mybir.dt.float16
MULT = mybir.AluOpType.mult
ADD = mybir.AluOpType.add
SUB = mybir.AluOpType.subtract
SIN = mybir.ActivationFunctionType.Sin
PI = float(np.pi)

_ENGINE_SEM_PREFIXES = {
    "DVE": ("DVE_",),
    "ACT": ("ACT_", "Activation_"),
    "PE": ("PE_",),
    "POOL": ("Pool_", "POOL_"),
    "SP": ("SP_",),
}


def strip_same_engine_waits(nc):
    for bb in nc.main_func.blocks:
        for ins in bb.instructions:
            si = getattr(ins, "sync_info", None)
            if si is None:
                continue
            eng = getattr(ins, "engine", None)
            pres = _ENGINE_SEM_PREFIXES.get(getattr(eng, "name", ""), ())
            if not pres:
                continue
            kept = [
                w
                for w in si.on_wait
                if not (
                    w.sync_type == "semaphore"
                    and w.ant_name
                    and w.ant_name.startswith(pres)
                )
            ]
            if len(kept) != len(si.on_wait):
                si.on_wait = kept
                ins.sync_info = si


def split_multi_waits(nc):
    """This walrus build allows one sync-wait per instruction: hoist extra
    waits onto same-engine Drain nops inserted just before the instruction."""
    for bb in nc.main_func.blocks:
        insts = bb.instructions
        i = 0
        while i < len(insts):
            ins = insts[i]
            si = getattr(ins, "sync_info", None)
            if si is None or len(si.on_wait) <= 1:
                i += 1
                continue
            waits = list(si.on_wait)
            for k, w in enumerate(waits[:-1]):
                d = mybir.InstDrain(name=f"{ins.name}_waitsplit{k}", ins=[], outs=[])
                d.engine = ins.engine
                d.sync_info = bass_rust.SyncInfo(on_wait=[w], on_update=[])
                insts.insert(i, d)
                i += 1
            si.on_wait = [waits[-1]]
            ins.sync_info = si
            i += 1


def fix_sync_waits(nc):
    split_multi_waits(nc)


def _skew_ap(strips, k, rb):
    """Dense row-major read of row-block rb of G_k (both planes) from the
    zero-backed strip area: element (r, c) of plane pl at strip offset
    r*SP + S0 + (c - r + W).  Dims: [r-part 128, pl 2, c 256]."""
    ap = strips[:]
    base = k * (2 * 256 * SP) + rb * 128 * (SP - 1) + (S0 + W)
    ap.ap = bass_rust.VecI64Pair([[SP - 1, 128], [256 * SP, 2], [1, 256]])
    ap.offset = base
    return ap


def build_nc(nsteps=GITS):
    nc = bass.Bass(num_devices=8)

    thg = nc.dram_tensor("thg", [18, N], F32, kind="ExternalInput")
    isg0 = nc.dram_tensor("isg0", [128, 1], F32, kind="ExternalInput")
    postm = nc.dram_tensor("postm", [128, 1], F32, kind="ExternalInput")
    gmd = nc.dram_tensor("gmd", [128, GITS], F32, kind="ExternalInput")
    gescd = nc.dram_tensor("gescd", [128, GITS, 2], F32, kind="ExternalInput")
    wconst = nc.dram_tensor("wconst", [4, 128, 128], F32, kind="ExternalInput")
    permw = nc.dram_tensor("permw", [4, 128, 128], F32, kind="ExternalInput")
    seld = nc.dram_tensor("seld", [128, 2, 32], F16, kind="ExternalInput")
    out_d = nc.dram_tensor("out", [128, 2, 64], F16, kind="ExternalOutput")

    JB = 68  # shipped band slice: packed j in [4, 4+JB)
    gsend = nc.dram_tensor("gsend", [256, 2, 68], F16, kind="Internal")
    gall = nc.dram_tensor("gall", [8, 256, 2, 68], F16, kind="Internal")
    strips = nc.dram_tensor("strips", [8, 2, 256, SP], F16, kind="Internal")

    with tile.TileContext(nc) as tc:
        with (
            tc.tile_pool(name="coef", bufs=1) as cp,
            tc.tile_pool(name="state", bufs=1) as sp,
            tc.tile_pool(name="mchain", bufs=2) as mp,
            tc.tile_pool(name="lts", bufs=2) as lp,
            tc.tile_pool(name="psum", bufs=2, space="PSUM") as pp,
        ):
            # ---------------- setup: trig + step coefficients ----------------
            th = cp.tile([128, 18, 2], F32, tag="th")
            Ct = cp.tile([128, 18, 2], F32, tag="Ct")
            St = cp.tile([128, 18, 2], F32, tag="St")
            wrk = cp.tile([128, 18, 2], F32, tag="wrk")
            wrp = cp.tile([128, 18, 2], F32, tag="wrp")
            zb = cp.tile([128, 1], F32, tag="zb")
            d1r = cp.tile([128, GITS, 2], F32, tag="d1r")
            d1i = cp.tile([128, GITS, 2], F32, tag="d1i")
            d1iN = cp.tile([128, GITS, 2], F32, tag="d1iN")
            d2r = cp.tile([128, GITS, 2], F32, tag="d2r")
            d2i = cp.tile([128, GITS, 2], F32, tag="d2i")
            isg = cp.tile([128, 1], F32, tag="isg")
            psm = cp.tile([128, 1], F32, tag="psm")
            gmt = cp.tile([128, GITS], F32, tag="gmt")
            gt = cp.tile([128, GITS, 2], F32, tag="gt")
            Wt = cp.tile([128, 4, 128], F32, tag="Wt")
            Pt = cp.tile([128, 4, 128], F32, tag="Pt")
            selt = cp.tile([128, 2, 32], F16, tag="selt")

            nc.sync.dma_start(th[:], thg[:].rearrange("it (k e) -> k it e", k=128, e=2))
            nc.sync.dma_start(isg[:], isg0[:])
            nc.sync.dma_start(psm[:], postm[:])
            nc.sync.dma_start(gmt[:], gmd[:])
            nc.sync.dma_start(gt[:], gescd[:])
            nc.sync.dma_start(Wt[:], wconst[:].rearrange("w p f -> p w f"))
            nc.sync.dma_start(Pt[:], permw[:].rearrange("w p f -> p w f"))
            nc.sync.dma_start(selt[:], seld[:])
            nc.vector.memset(zb[:], 0.0)

            # zero-fill the skew strips (after the inputs so they don't
            # delay setup; overlaps phase A on the DMA engines)
            zt = cp.tile([128, 2048], F16, tag="zt")
            nc.vector.memset(zt[:], 0.0)
            for g in range(8):
                dst = strips[g].rearrange(
                    "pl (pa pb) c -> (pl pa) (pb c)", pa=64, pb=4
                )
                nc.gpsimd.dma_start(dst, zt[:])

            # sin/cos with range reduction into (-pi, pi]
            nc.vector.tensor_scalar(wrp[:], th[:], PI, -2 * PI, mybir.AluOpType.is_gt, MULT)
            nc.vector.tensor_tensor(wrk[:], th[:], wrp[:], ADD)
            nc.scalar.activation(St[:], wrk[:], SIN, bias=zb[:])
            nc.vector.tensor_scalar(wrk[:], th[:], PI / 2, None, ADD)
            nc.vector.tensor_scalar(wrp[:], wrk[:], PI, -2 * PI, mybir.AluOpType.is_gt, MULT)
            nc.vector.tensor_tensor(wrk[:], wrk[:], wrp[:], ADD)
            nc.scalar.activation(Ct[:], wrk[:], SIN, bias=zb[:])

            Cmid = Ct[:, :GITS, :]
            Smid = St[:, :GITS, :]
            Csw = Ct[:, :GITS, ::-1]
            Ssw = St[:, :GITS, ::-1]
            wmid = wrk[:, :GITS, :]

            # d1 = at^2 p - ar^2 p^sigma ; d2 = i at ar (p + p^sigma)
            nc.vector.tensor_scalar(wmid, Csw, -AR * AR, None, MULT)
            nc.vector.scalar_tensor_tensor(d1r[:], Cmid, AT * AT, wmid, MULT, ADD)
            nc.vector.tensor_scalar(wmid, Ssw, -AR * AR, None, MULT)
            nc.vector.scalar_tensor_tensor(d1i[:], Smid, AT * AT, wmid, MULT, ADD)
            nc.vector.tensor_scalar(d1iN[:], d1i[:], -1.0, None, MULT)
            nc.vector.tensor_tensor(wmid, Smid, Ssw, ADD)
            nc.vector.tensor_scalar(d2r[:], wmid, -AT * AR, None, MULT)
            nc.vector.tensor_tensor(wmid, Cmid, Csw, ADD)
            nc.vector.tensor_scalar(d2i[:], wmid, AT * AR, None, MULT)

            # p129 post-scale blend: ceff = postm*c129 + (1-postm); seff = postm*s129
            ceff = cp.tile([128, 2], F32, tag="ceff")
            seff = cp.tile([128, 2], F32, tag="seff")
            seffN = cp.tile([128, 2], F32, tag="seffN")
            npsm = cp.tile([128, 1], F32, tag="npsm")
            nc.vector.tensor_scalar(npsm[:], psm[:], -1.0, 1.0, MULT, ADD)
            for e in range(2):
                nc.vector.scalar_tensor_tensor(
                    ceff[:, e : e + 1], Ct[:, 17, e : e + 1], psm[:], npsm[:], MULT, ADD
                )
                nc.vector.tensor_scalar(seff[:, e : e + 1], St[:, 17, e : e + 1], psm[:], None, MULT)
            nc.vector.tensor_scalar(seffN[:], seff[:], -1.0, None, MULT)

            # ---------------- phase A state ----------------
            Gpp_a = sp.tile([128, 2, 2, J], F32, tag="Ga")
            Gpp_b = sp.tile([128, 2, 2, J], F32, tag="Gb")
            Gpp = [Gpp_a, Gpp_b]
            G = Gpp[0]
            V = sp.tile([128, 2, 2, J], F32, tag="V")
            Ypp_a = sp.tile([128, 2, 2, J], F32, tag="Ya")
            Ypp_b = sp.tile([128, 2, 2, J], F32, tag="Yb")
            Ypp = [Ypp_a, Ypp_b]
            Y = Ypp[0]
            U = sp.tile([128, 2, 2, J], F32, tag="U")
            Ysh = sp.tile([128, 2, 2, 80], F32, tag="Ysh")
            TGT = sp.tile([128, 2, 2, J], F32, tag="TGT")
            T0 = sp.tile([128, 2, J], F32, tag="T0")
            T1 = sp.tile([128, 2, J], F32, tag="T1")
            for i in range(2):
                nc.vector.memset(Gpp[i][:], 0.0)
                nc.vector.memset(Ypp[i][:], 0.0)
            nc.vector.memset(V[:], 0.0)
            nc.vector.memset(U[:], 0.0)

            # G init: identity (or diag(p0) on core 0): G[p,e,:,W]
            nm0 = cp.tile([128, 1], F32, tag="nm0")
            nc.vector.tensor_scalar(nm0[:], isg[:], -1.0, 1.0, MULT, ADD)
            for e in range(2):
                nc.vector.scalar_tensor_tensor(
                    G[:, e, 0, W : W + 1], Ct[:, 16, e : e + 1], isg[:], nm0[:], MULT, ADD
                )
                nc.vector.tensor_scalar(
                    G[:, e, 1, W : W + 1], St[:, 16, e : e + 1], isg[:], None, MULT
                )


            # ---------------- phase A: 16 group steps ----------------
            for s in range(nsteps):
                G = Gpp[s % 2]
                Gout = Gpp[(s + 1) % 2]
                Y = Ypp[s % 2]
                he = 2 * s + 1   # E-step half-width
                hc = 2 * s + 2   # CR half-width

                def sl(h, d=0):
                    return slice(W - h + d, W + h + 1 + d)

                cd1r = [d1r[:, s, e : e + 1] for e in range(2)]
                cd1i = [d1i[:, s, e : e + 1] for e in range(2)]
                cd1iN = [d1iN[:, s, e : e + 1] for e in range(2)]
                cd2r = d2r[:, s, 0:1]
                cd2i = d2i[:, s, 0:1]

                # V = i*G on the CR window (reads beyond G's band are zeros)
                nc.vector.tensor_scalar(V[:, :, 0, sl(hc)], G[:, :, 1, sl(hc)], -1.0, None, MULT)
                nc.vector.tensor_copy(out=V[:, :, 1, sl(hc)], in_=G[:, :, 0, sl(hc)])

                # u[e] = d2 * G[1-e] (j-shifted -1 for e=0, +1 for e=1)
                nc.vector.tensor_scalar(T0[:, :, sl(he)], V[:, 1, :, sl(he, -1)], cd2i, None, MULT)
                nc.vector.scalar_tensor_tensor(
                    U[:, 0, :, sl(he)], G[:, 1, :, sl(he, -1)], cd2r, T0[:, :, sl(he)], MULT, ADD
                )
                nc.vector.tensor_scalar(T1[:, :, sl(he)], V[:, 0, :, sl(he, +1)], cd2i, None, MULT)
                nc.vector.scalar_tensor_tensor(
                    U[:, 1, :, sl(he)], G[:, 0, :, sl(he, +1)], cd2r, T1[:, :, sl(he)], MULT, ADD
                )

                # Y[e] = d1r[e]*G[e] + d1i[e]*V[e] + u[e]  (two-plane fused).
                # Stored pre-shifted per e (e=0 at j-1, e=1 at j+1) so the
                # crossing matmuls read unshifted 16B-aligned windows.
                for e in range(2):
                    sh = +1 if e == 0 else -1
                    nc.vector.scalar_tensor_tensor(
                        Y[:, e, :, sl(he, sh)], V[:, e, :, sl(he)], cd1i[e], U[:, e, :, sl(he)], MULT, ADD
                    )
                    nc.vector.scalar_tensor_tensor(
                        Y[:, e, :, sl(he, sh)], G[:, e, :, sl(he)], cd1r[e], Y[:, e, :, sl(he, sh)], MULT, ADD
                    )

                # CR: sgP[p,0] = (+-G2C)*Y[p-1,1] at j+1; sgP[p,1] = (+-G2C)*Y[p+1,0] at j-1
                # Shift on DVE (any offset), then matmul on 16B-aligned windows.
                lo4 = ((W - hc) // 4) * 4
                hi4 = ((W + hc + 1 + 3) // 4) * 4
                al = slice(lo4, hi4)
                alp = slice(lo4 + 1, hi4 + 1)
                alm = slice(lo4 - 1, hi4 - 1)
                sgP = pp.tile([128, 2, 2, J], F32, tag="sgP")
                nc.tensor.matmul(sgP[:, 0, 0, al], Wt[:, 0, :], Y[:, 1, 1, al], start=True, stop=True)
                nc.tensor.matmul(sgP[:, 0, 1, al], Wt[:, 1, :], Y[:, 1, 0, al], start=True, stop=True)
                nc.tensor.matmul(sgP[:, 1, 0, al], Wt[:, 2, :], Y[:, 0, 1, al], start=True, stop=True)
                nc.tensor.matmul(sgP[:, 1, 1, al], Wt[:, 3, :], Y[:, 0, 0, al], start=True, stop=True)

                # tg[e] = gesc[s,e] * Y[e]  (ACT, off critical path)
                nc.vector.tensor_scalar(TGT[:, 0, :, sl(hc)], Y[:, 0, :, sl(hc, +1)], gt[:, s, 0:1], None, MULT)
                nc.scalar.mul(TGT[:, 1, :, sl(hc)], Y[:, 1, :, sl(hc, -1)], gt[:, s, 1:2])

                # G' = gm[s]*sgP + tg  (single 4-plane op; esc lives in TGT)
                nc.vector.scalar_tensor_tensor(
                    Gout[:, :, :, sl(hc)], sgP[:, :, :, sl(hc)], gmt[:, s : s + 1],
                    TGT[:, :, :, sl(hc)], MULT, ADD,
                )

            # ---------------- p129 post-scale (core 7; identity elsewhere) ---
            G = Gpp[nsteps % 2]
            fb = slice(W - 32, W + 33)
            for e in range(2):
                nc.vector.tensor_scalar(T0[:, e, fb], G[:, e, 1, fb], seffN[:, e : e + 1], None, MULT)
                nc.vector.tensor_scalar(T1[:, e, fb], G[:, e, 1, fb], ceff[:, e : e + 1], None, MULT)
                nc.vector.scalar_tensor_tensor(
                    G[:, e, 1, fb], G[:, e, 0, fb], seff[:, e : e + 1], T1[:, e, fb], MULT, ADD
                )
                nc.vector.scalar_tensor_tensor(
                    G[:, e, 0, fb], G[:, e, 0, fb], ceff[:, e : e + 1], T0[:, e, fb], MULT, ADD
                )


            # ---------------- pair -> linear row permutation ----------------
            pp2 = pp.tile([128, 2, 2, J], F32, tag="pp2", bufs=1)
            for rb in range(2):
                nc.tensor.matmul(pp2[:, rb, :, :], Pt[:, 2 * rb + 0, :], G[:, 0, :, :], start=True, stop=False)
                nc.tensor.matmul(pp2[:, rb, :, :], Pt[:, 2 * rb + 1, :], G[:, 1, :, :], start=False, stop=True)
            glin = sp.tile([128, 2, 2, 68], F16, tag="glin")
            nc.vector.tensor_copy(out=glin[:], in_=pp2[:, :, :, 4:72])

            # ---------------- stage out + AllGather + strip scatter ----------
            nc.sync.dma_start(
                gsend[:].rearrange("(rb p) ri j -> p rb ri j", rb=2, p=128),
                glin[:],
            )
            nc.gpsimd.collective_compute(
                "AllGather", mybir.AluOpType.bypass,
                replica_groups=[[0, 1, 2, 3, 4, 5, 6, 7]],
                ins=[gsend[:]],
                outs=[gall[:]],
            )
            ORDER = (7, 6, 5, 4, 3, 2, 1, 0)  # chain consumption order

            # ---------------- column-sharded F^T chain ------------------------
            # Core m computes F^T[:, 32m:32m+32] = G0^T ... G7^T @ SEL where
            # SEL is its one-hot column selector (per-core input data; the
            # program is uniform). The host assembles the full 256x256 output
            # from the 8 cores' 32-column slices. M is packed [p, ab, R|I]
            # (64 free) so one ltR matmul advances both planes at once.

            # keep PE clocked up through the collective: dummy matmuls on
            # stale data (PE is otherwise idle and would drop to the low
            # pstate, halving chain matmul speed)
            warm = pp.tile([128, 2, 2, J], F32, tag="sgP")
            for _w in range(60):
                nc.tensor.matmul(
                    warm[:].rearrange('p a b c -> p (a b c)')[:, 0:256],
                    Pt[:, 0, :],
                    Pt[:, 0:2, :].rearrange('p w f -> p (w f)'),
                    start=True, stop=True,
                )

            lts = [None] * 8
            ltIns = [None] * 8
            for idx, k in enumerate(ORDER):
                q = nc.sync if idx % 2 == 0 else nc.scalar
                q.dma_start(
                    strips[k, :, :, S0 + 4 : S0 + 4 + 68],
                    gall[k].rearrange("r pl j -> pl r j"),
                )
                ltk = lp.tile([128, 2, 2, 256], F16, tag=f"ltk{k}", bufs=1)
                for rb in range(2):
                    q.dma_start(ltk[:, rb, :, :], _skew_ap(strips, k, rb))
                lts[k] = ltk
                lik = lp.tile([128, 2, 256], F16, tag=f"ltIn{k}", bufs=1)
                nc.gpsimd.tensor_scalar(lik[:], ltk[:, :, 1, :], -1.0, None, MULT)
                ltIns[k] = lik

            # chain: M <- G_k^T @ M for k = 7..0, starting M = SEL (real)
            M = None
            for idx, k in enumerate(ORDER):
                lt = lts[k]
                lik = ltIns[k]
                P = pp.tile([128, 2, 64], F32, tag=f"P{idx % 2}", bufs=1)
                for ab in range(2):
                    abv = slice(128 * ab, 128 * (ab + 1))
                    if idx == 0:
                        # M = SEL: real one-hot; P.R = ltR^T SEL, P.I = ltI^T SEL
                        nc.tensor.matmul(P[:, ab, 0:32], lt[:, 0, 0, abv], selt[:, 0, :], start=True, stop=False)
                        nc.tensor.matmul(P[:, ab, 0:32], lt[:, 1, 0, abv], selt[:, 1, :], start=False, stop=False)
                        nc.tensor.matmul(P[:, ab, 32:64], lt[:, 0, 1, abv], selt[:, 0, :], start=False, stop=False, skip_group_check=True)
                        nc.tensor.matmul(P[:, ab, 32:64], lt[:, 1, 1, abv], selt[:, 1, :], start=False, stop=True, skip_group_check=True)
                    else:
                        # full-range ltR matmuls first (start zeroes the bank),
                        # then partial-range complex cross terms accumulate
                        nc.tensor.matmul(P[:, ab, :], lt[:, 0, 0, abv], M[:, 0, :], start=True, stop=False)
                        nc.tensor.matmul(P[:, ab, :], lt[:, 1, 0, abv], M[:, 1, :], start=False, stop=False)
                        nc.tensor.matmul(P[:, ab, 0:32], lik[:, 0, abv], M[:, 0, 32:64], start=False, stop=False, skip_group_check=True)
                        nc.tensor.matmul(P[:, ab, 0:32], lik[:, 1, abv], M[:, 1, 32:64], start=False, stop=False, skip_group_check=True)
                        nc.tensor.matmul(P[:, ab, 32:64], lt[:, 0, 1, abv], M[:, 0, 0:32], start=False, stop=False, skip_group_check=True)
                        nc.tensor.matmul(P[:, ab, 32:64], lt[:, 1, 1, abv], M[:, 1, 0:32], start=False, stop=True, skip_group_check=True)
                Mn = mp.tile([128, 2, 64], F16, tag=f"M{idx % 2}")
                for ab in range(2):
                    nc.vector.tensor_copy(out=Mn[:, ab, :], in_=P[:, ab, :])
                M = Mn

            nc.sync.dma_start(out_d[:], M[:])


    return nc


def make_inputs(core: int, thetas: np.ndarray):
    m = core
    thg = np.concatenate(
        [thetas[16 * m + 1 : 16 * m + 17], thetas[0:1], thetas[129:130]], axis=0
    ).astype(np.float32)
    isg0 = np.full((128, 1), 1.0 if m == 0 else 0.0, np.float32)
    postm = np.full((128, 1), 1.0 if m == 7 else 0.0, np.float32)
    gm = np.ones((128, GITS), np.float32)
    gesc = np.full((128, GITS, 2), G1S, np.float32)
    gesc[0, :, 0] = G2C
    gesc[127, :, 1] = G2C
    if m == 7:
        gm[:, GITS - 1] = 0.0
        gesc[:, GITS - 1, :] = 1.0

    wdn = np.eye(128, k=1, dtype=np.float32)
    wup = np.eye(128, k=-1, dtype=np.float32)
    wconst = np.stack([-G2C * wdn, G2C * wdn, -G2C * wup, G2C * wup]).astype(np.float32)

    permw = np.zeros((4, 128, 128), np.float32)
    for rb in range(2):
        for e in range(2):
            for p in range(64 * rb, 64 * rb + 64):
                permw[2 * rb + e, p, 2 * p + e - 128 * rb] = 1.0

    sel = np.zeros((128, 2, 32), np.float16)
    for j in range(32):
        r = 32 * m + j
        sel[r % 128, r // 128, j] = 1.0

    return {
        "thg": thg, "isg0": isg0, "postm": postm, "gmd": gm, "gescd": gesc,
        "wconst": wconst, "permw": permw, "seld": sel,
    }


_CACHE = {}


def _get_nc():
    if "nc" not in _CACHE:
        nc = build_nc()
        fix_sync_waits(nc)
        _CACHE["nc"] = nc
    return _CACHE["nc"]


def _run(thetas: np.ndarray, trace: bool = False):
    thetas = np.ascontiguousarray(thetas, dtype=np.float32)
    assert thetas.shape == (130, N)
    nc = _get_nc()
    in_maps = [make_inputs(c, thetas) for c in range(NCORES)]
    res = run_bass_kernel_spmd(nc, in_maps, list(range(NCORES)), trace=trace)
    # core m's out [128, 2(ab), 64(R|I)] holds F^T[:, 32m:32m+32]
    FT = np.empty((256, 256), np.complex64)
    for m in range(NCORES):
        o = res.results[m]["out"]
        sl_ = (o[:, :, 0:32] + 1j * o[:, :, 32:64]).astype(np.complex64)
        FT[:, 32 * m : 32 * m + 32] = sl_.transpose(1, 0, 2).reshape(256, 32)
    return FT.T.copy(), res


def kernel(thetas: np.ndarray) -> np.ndarray:
    out, _ = _run(thetas, trace=False)
    return out

